# revision 1
# baseline (speedup 1.0000x reference)
"""GNN DifferentiableGraphGenerator Trainium2 kernel (dev version).

Per-core: 6250 nodes x 15 candidate edges, fp32 energy path.
Gather strategy: per half-core compacted C-table (distinct src rows ~30.5k
< 32768 so signed-int16 dma_gather indices reach everything), table built
on-device by streaming host-reordered x^T columns through PE.
"""
import numpy as np
from contextlib import ExitStack

import concourse.bass as bass
import concourse.tile as tile
from concourse import bacc, mybir
from concourse.masks import make_identity

F32 = mybir.dt.float32
I16 = mybir.dt.int16
AX = mybir.AxisListType
OP = mybir.AluOpType
ACTF = mybir.ActivationFunctionType

N = 50000
K = 15
D = 128
H = 64
NC_CORES = 8
N_CORE = N // NC_CORES          # 6250
NP = 128                        # nodes per edge-tile (partition dim)
NT = 49                         # edge tiles per core (48 full + 1 partial)
NPAD = NP * NT                  # 6272 padded nodes per core
LH = 31250                      # compacted table rows per half (host-padded)
CH = LH // 125                  # 250 chunks of 125 per half-table
E_TILE = NP * K                 # 1920
HALF_T = 24                     # tiles 0..23 -> table A, 24..48 -> table B
TAU = 0.5


def mk_ap(ap, offset, dims):
    return bass.AP(ap.tensor, offset, dims)


def build_kernel(meta):
    nc = bacc.Bacc("TRN2", target_bir_lowering=False, debug=False,
                   enable_asserts=False, num_devices=NC_CORES)

    # ---- DRAM I/O (per-core tensors) ----
    xta = nc.dram_tensor("xta", [D, LH], F32, kind="ExternalInput").ap()
    xtb = nc.dram_tensor("xtb", [D, LH], F32, kind="ExternalInput").ap()
    xT_own = nc.dram_tensor("xT_own", [D, NPAD], F32, kind="ExternalInput").ap()
    srcidx = nc.dram_tensor("srcidx", [D, NT * (E_TILE // 16)], I16,
                            kind="ExternalInput").ap()
    dist = nc.dram_tensor("dist", [NP, NT * K], F32, kind="ExternalInput").ap()
    wsrc = nc.dram_tensor("wsrc", [D, H], F32, kind="ExternalInput").ap()
    wdst = nc.dram_tensor("wdst", [D, H], F32, kind="ExternalInput").ap()
    wdist_d = nc.dram_tensor("wdist", [1, H], F32, kind="ExternalInput").ap()
    eb1_d = nc.dram_tensor("eb1", [1, H], F32, kind="ExternalInput").ap()
    wblk = nc.dram_tensor("wblk", [2 * H, 2 * H], F32, kind="ExternalInput").ap()
    b2p_d = nc.dram_tensor("b2p", [2 * H, 1], F32, kind="ExternalInput").ap()
    dw1a = nc.dram_tensor("dw1a", [D, H], F32, kind="ExternalInput").ap()
    dw1b_d = nc.dram_tensor("dw1b", [1, H], F32, kind="ExternalInput").ap()
    db1_d = nc.dram_tensor("db1", [1, H], F32, kind="ExternalInput").ap()
    w2d = nc.dram_tensor("w2d", [H, H], F32, kind="ExternalInput").ap()
    b2d_d = nc.dram_tensor("b2d", [H, 1], F32, kind="ExternalInput").ap()

    out_w = nc.dram_tensor("out_w", [NP, NT * K], F32, kind="ExternalOutput").ap()
    out_g = nc.dram_tensor("out_g", [NP, NT * K], F32, kind="ExternalOutput").ap()
    out_e = nc.dram_tensor("out_e", [NP, NT * K], F32, kind="ExternalOutput").ap()
    out_k = nc.dram_tensor("out_k", [NP, NT], F32, kind="ExternalOutput").ap()

    # compacted C tables (row layout: table row (q%125)*CH + q//125 for
    # column q of the corresponding xT input; srcidx already encodes this)
    C_a = nc.dram_tensor("C_a", [LH, H], F32, kind="Internal").ap()
    C_b = nc.dram_tensor("C_b", [LH, H], F32, kind="Internal").ap()

    mpos = meta["mpos"]
    mposd = meta["mposd"]
    b3 = float(meta["b3"])
    db3 = float(meta["db3"])
    thr = [float(t) for t in meta["thr"]]

    with tile.TileContext(nc) as tc, ExitStack() as ctx:
        # ---------------- pools (PSUM: cps 2 + tp 2 + l2 2 + bp 2 = 8) -----
        const_p = ctx.enter_context(tc.tile_pool(name="const", bufs=1))
        xin_p = ctx.enter_context(tc.tile_pool(name="xin", bufs=2))
        cpsum_p = ctx.enter_context(tc.tile_pool(name="cpsum", bufs=2, space="PSUM"))
        cstage_p = ctx.enter_context(tc.tile_pool(name="cstage", bufs=2))
        res_p = ctx.enter_context(tc.tile_pool(name="res", bufs=1))
        pre_p = ctx.enter_context(tc.tile_pool(name="pre", bufs=2))
        gat_p = ctx.enter_context(tc.tile_pool(name="gat", bufs=2))
        tpsum_p = ctx.enter_context(tc.tile_pool(name="tpsum", bufs=1, space="PSUM"))
        h1_p = ctx.enter_context(tc.tile_pool(name="h1", bufs=2))
        l2psum_p = ctx.enter_context(tc.tile_pool(name="l2psum", bufs=1, space="PSUM"))
        h2_p = ctx.enter_context(tc.tile_pool(name="h2", bufs=2))
        bpsum_p = ctx.enter_context(tc.tile_pool(name="bpsum", bufs=1, space="PSUM"))
        fin_p = ctx.enter_context(tc.tile_pool(name="fin", bufs=1))
        dg_p = ctx.enter_context(tc.tile_pool(name="dg", bufs=2))

        # ---------------- constants ----------------
        ident = const_p.tile([D, D], F32)
        make_identity(nc, ident[:])
        wsrc_sb = const_p.tile([D, H], F32)
        nc.sync.dma_start(wsrc_sb[:], wsrc[:, :])
        wdst_sb = const_p.tile([D, H], F32)
        nc.sync.dma_start(wdst_sb[:], wdst[:, :])
        dw1a_sb = const_p.tile([D, H], F32)
        nc.sync.dma_start(dw1a_sb[:], dw1a[:, :])
        wblk_sb = const_p.tile([2 * H, 2 * H], F32)
        nc.sync.dma_start(wblk_sb[:], wblk[:, :])
        w2d_sb = const_p.tile([H, H], F32)
        nc.sync.dma_start(w2d_sb[:], w2d[:, :])
        b2p_sb = const_p.tile([2 * H, 1], F32)
        nc.sync.dma_start(b2p_sb[:], b2p_d[:, :])
        b2d_sb = const_p.tile([H, 1], F32)
        nc.sync.dma_start(b2d_sb[:], b2d_d[:, :])
        wdist_t = const_p.tile([NP, H], F32)
        nc.sync.dma_start(wdist_t[:], mk_ap(wdist_d, 0, [[0, NP], [1, H]]))
        eb1_t = const_p.tile([NP, H], F32)
        nc.sync.dma_start(eb1_t[:], mk_ap(eb1_d, 0, [[0, NP], [1, H]]))
        dw1b_t = const_p.tile([NP, H], F32)
        nc.sync.dma_start(dw1b_t[:], mk_ap(dw1b_d, 0, [[0, NP], [1, H]]))
        db1_t = const_p.tile([NP, H], F32)
        nc.sync.dma_start(db1_t[:], mk_ap(db1_d, 0, [[0, NP], [1, H]]))

        soff_sb = res_p.tile([D, NT * (E_TILE // 16)], I16)
        nc.sync.dma_start(soff_sb[:], srcidx[:, :])
        dist_sb = res_p.tile([NP, NT * K], F32)
        nc.sync.dma_start(dist_sb[:], dist[:, :])
        A_res = res_p.tile([NP, NT * H], F32)
        D_res = res_p.tile([NP, NT * H], F32)
        L_res = res_p.tile([NP, NT * K], F32)
        E_res = res_p.tile([NP, NT * K], F32)

        # ---------------- P1: compacted C tables ----------------
        XCOLS = 3125   # 25 chunks of 125 per block; 10 blocks per half
        CSTAGE = 25
        for half, (xt_in, c_out) in enumerate(((xta, C_a), (xtb, C_b))):
            for blk in range(LH // XCOLS):
                xt = xin_p.tile([D, XCOLS], F32, tag="xt")
                nc.sync.dma_start(xt[:], xt_in[:, blk * XCOLS:(blk + 1) * XCOLS])
                cst = cstage_p.tile([125, CSTAGE * H], F32, tag="cst")
                for s in range(CSTAGE):
                    cps = cpsum_p.tile([128, 500], F32, space="PSUM", tag="cps")
                    lhs = xt[:, s * 125:(s + 1) * 125]
                    nc.tensor.matmul(cps[:125, :H], lhs, wsrc_sb[:],
                                     start=True, stop=True)
                    nc.any.tensor_copy(cst[:, s * H:(s + 1) * H], cps[:125, :H])
                c0 = blk * CSTAGE
                dst = mk_ap(c_out, c0 * H, [[CH * H, 125], [H, CSTAGE], [1, H]])
                nc.sync.dma_start(dst, cst[:].rearrange("p (s f) -> p s f", f=H))

        # ---------------- P1b: A and D for own slice ----------------
        for t in range(NT):
            xo = xin_p.tile([D, XCOLS], F32, tag="xt")
            nc.sync.dma_start(xo[:, :NP], xT_own[:, t * NP:(t + 1) * NP])
            lhs = xo[:, :NP]
            aps = cpsum_p.tile([128, 500], F32, space="PSUM", tag="cps")
            nc.tensor.matmul(aps[:, :H], lhs, wdst_sb[:], start=True, stop=True)
            nc.vector.tensor_add(A_res[:, t * H:(t + 1) * H], aps[:, :H], eb1_t[:])
            dps = cpsum_p.tile([128, 500], F32, space="PSUM", tag="cps")
            nc.tensor.matmul(dps[:, :H], lhs, dw1a_sb[:], start=True, stop=True)
            nc.vector.tensor_add(D_res[:, t * H:(t + 1) * H], dps[:, :H], db1_t[:])

        # ---------------- P2: edge pipeline ----------------
        NPAIR = 7
        NIT = E_TILE // 16  # 120 idx columns per tile
        for t in range(NT):
            c_tab = C_a if t < HALF_T else C_b
            gat = gat_p.tile([NP, K * H], F32, tag="gat")
            nc.gpsimd.dma_gather(
                out_ap=gat[:].rearrange("p (j f) -> p j f", f=H),
                in_ap=c_tab[:, :],
                idxs_ap=soff_sb[:, t * NIT:(t + 1) * NIT],
                num_idxs=E_TILE, num_idxs_reg=E_TILE, elem_size=H,
                single_packet=False)

            pre = pre_p.tile([NP, K * H], F32, tag="pre")
            pre3 = pre[:].rearrange("p (j f) -> p j f", f=H)
            wv = mk_ap(wdist_t[:], wdist_t[:].offset,
                       [wdist_t[:].ap[0], [0, K], [1, H]])
            dv = dist_sb[:, t * K:(t + 1) * K].unsqueeze(2).broadcast_to([NP, K, H])
            nc.vector.tensor_tensor(out=pre3, in0=wv, in1=dv, op=OP.mult)
            av = A_res[:, t * H:(t + 1) * H].unsqueeze(1).broadcast_to([NP, K, H])
            nc.vector.tensor_tensor(out=pre3, in0=pre3, in1=av, op=OP.add)
            nc.vector.tensor_add(pre[:], pre[:], gat[:])

            # transposes -> bank-aligned stacked fm: pairs 0-3 at q*128,
            # 4-6 at 512+(q-4)*128, leftover slot14 at 896:1024 rows 0:H
            def pcol(q):
                return q * NP if q < 4 else 512 + (q - 4) * NP
            LCq = 896
            tp = tpsum_p.tile([2 * H, 1024], F32, space="PSUM", tag="tp")
            for q in range(NPAIR):
                nc.tensor.transpose(tp[:, pcol(q):pcol(q) + NP],
                                    pre[:, 2 * q * H:(2 * q + 2) * H], ident[:])
            nc.tensor.transpose(tp[:H, LCq:LCq + NP], pre[:, 14 * H:15 * H],
                                ident[:])

            h1 = h1_p.tile([2 * H, 1024], F32, tag="h1")
            nc.any.tensor_relu(h1[:, 0:512], tp[:, 0:512])
            nc.any.tensor_relu(h1[:, 512:LCq], tp[:, 512:LCq])
            nc.any.tensor_relu(h1[:H, LCq:], tp[:H, LCq:])

            l2 = l2psum_p.tile([2 * H, 1024], F32, space="PSUM", tag="l2")
            nc.tensor.matmul(l2[:, 0:512], wblk_sb[:], h1[:, 0:512],
                             start=True, stop=True)
            nc.tensor.matmul(l2[:, 512:LCq], wblk_sb[:], h1[:, 512:LCq],
                             start=True, stop=True)
            nc.tensor.matmul(l2[:H, LCq:], wblk_sb[:H, :H], h1[:H, LCq:],
                             start=True, stop=True)

            h2 = h2_p.tile([2 * H, 1024], F32, tag="h2")
            nc.scalar.activation(h2[:, 0:512], l2[:, 0:512], ACTF.Relu,
                                 bias=b2p_sb[:, 0:1])
            nc.scalar.activation(h2[:, 512:LCq], l2[:, 512:LCq], ACTF.Relu,
                                 bias=b2p_sb[:, 0:1])
            nc.scalar.activation(h2[:H, LCq:], l2[:H, LCq:], ACTF.Relu,
                                 bias=b2p_sb[:H, 0:1])

            bp = bpsum_p.tile([NP, K * H], F32, space="PSUM", tag="bp")
            for q in range(NPAIR):
                nc.tensor.transpose(bp[:, 2 * q * H:(2 * q + 2) * H],
                                    h2[:, pcol(q):pcol(q) + NP], ident[:])
            nc.tensor.transpose(bp[:, 14 * H:15 * H], h2[:H, LCq:LCq + NP],
                                ident[:H, :H])

            pos_m = mk_ap(bp[:], bp[:].offset,
                          [bp[:].ap[0], [2 * H, NPAIR], [H, 2], [1, mpos]])
            neg_m = mk_ap(bp[:], bp[:].offset + mpos,
                          [bp[:].ap[0], [2 * H, NPAIR], [H, 2], [1, H - mpos]])
            posl = mk_ap(bp[:], bp[:].offset + 14 * H, [bp[:].ap[0], [1, 1], [1, mpos]])
            negl = mk_ap(bp[:], bp[:].offset + 14 * H + mpos,
                         [bp[:].ap[0], [1, 1], [1, H - mpos]])
            psum_t = fin_p.tile([NP, K], F32, tag="psum_t")
            nsum_t = fin_p.tile([NP, K], F32, tag="nsum_t")
            nc.vector.tensor_reduce(psum_t[:, 0:14].rearrange("p (q h) -> p q h", h=2),
                                    pos_m, axis=AX.X, op=OP.add)
            nc.vector.tensor_reduce(nsum_t[:, 0:14].rearrange("p (q h) -> p q h", h=2),
                                    neg_m, axis=AX.X, op=OP.add)
            nc.vector.tensor_reduce(psum_t[:, 14:15], posl, axis=AX.X, op=OP.add)
            nc.vector.tensor_reduce(nsum_t[:, 14:15], negl, axis=AX.X, op=OP.add)
            nc.vector.tensor_sub(L_res[:, t * K:(t + 1) * K], psum_t[:], nsum_t[:])
            nc.scalar.activation(E_res[:, t * K:(t + 1) * K],
                                 L_res[:, t * K:(t + 1) * K],
                                 ACTF.Sigmoid, bias=2.0 * b3, scale=2.0)

        # ---------------- P3: ranking / degree / outputs ----------------
        Lr = L_res[:].rearrange("p (t j) -> p t j", j=K)
        cnt = fin_p.tile([NP, NT * K], F32, tag="cnt")
        tmp = fin_p.tile([NP, NT * K], F32, tag="tmp")
        for j in range(K):
            bj = mk_ap(L_res[:], L_res[:].offset + j,
                       [L_res[:].ap[0], [K, NT], [0, K]])
            if j == 0:
                nc.vector.tensor_tensor(out=cnt[:].rearrange("p (t j) -> p t j", j=K),
                                        in0=bj, in1=Lr, op=OP.is_gt)
            else:
                nc.vector.tensor_tensor(out=tmp[:].rearrange("p (t j) -> p t j", j=K),
                                        in0=bj, in1=Lr, op=OP.is_gt)
                nc.vector.tensor_add(cnt[:], cnt[:], tmp[:])
        for j in range(K - 1):
            w = K - 1 - j
            bj = mk_ap(L_res[:], L_res[:].offset + j,
                       [L_res[:].ap[0], [K, NT], [0, w]])
            sfx = mk_ap(L_res[:], L_res[:].offset + j + 1,
                        [L_res[:].ap[0], [K, NT], [1, w]])
            tv = mk_ap(tmp[:], tmp[:].offset + j + 1,
                       [tmp[:].ap[0], [K, NT], [1, w]])
            cv = mk_ap(cnt[:], cnt[:].offset + j + 1,
                       [cnt[:].ap[0], [K, NT], [1, w]])
            nc.vector.tensor_tensor(out=tv, in0=bj, in1=sfx, op=OP.is_equal)
            nc.vector.tensor_tensor(out=cv, in0=cv, in1=tv, op=OP.add)
        # cnt = rank-1 (0-based)

        hint = dg_p.tile([NP, NT], F32, tag="hint")
        nc.vector.tensor_reduce(hint[:], E_res[:].rearrange("p (t j) -> p t j", j=K),
                                axis=AX.X, op=OP.add)
        g1pre = dg_p.tile([NP, NT * H], F32, tag="g1pre")
        hv = hint[:].unsqueeze(2).broadcast_to([NP, NT, H])
        wv1 = mk_ap(dw1b_t[:], dw1b_t[:].offset, [dw1b_t[:].ap[0], [0, NT], [1, H]])
        nc.vector.tensor_tensor(out=g1pre[:].rearrange("p (t f) -> p t f", f=H),
                                in0=hv, in1=wv1, op=OP.mult)
        nc.vector.tensor_add(g1pre[:], g1pre[:], D_res[:])
        nc.any.tensor_relu(g1pre[:], g1pre[:])
        g1T = dg_p.tile([H, NPAD], F32, tag="g1T")
        for t in range(NT):
            gps = cpsum_p.tile([128, 500], F32, space="PSUM", tag="cps")
            nc.tensor.transpose(gps[:H, :NP], g1pre[:, t * H:(t + 1) * H], ident[:])
            nc.any.tensor_copy(g1T[:, t * NP:(t + 1) * NP], gps[:H, :NP])
        kraw = dg_p.tile([NP, NT], F32, tag="kraw")
        kps_t = dg_p.tile([NP, NT], F32, tag="kps")
        kns_t = dg_p.tile([NP, NT], F32, tag="kns")
        DN = 128  # one node-tile per chunk
        for b in range(NPAD // DN):
            zps = cpsum_p.tile([128, 500], F32, space="PSUM", tag="cps")
            g2c = dg_p.tile([H, DN], F32, tag="g2c")
            nc.tensor.matmul(zps[:H, :DN], w2d_sb[:], g1T[:, b * DN:(b + 1) * DN],
                             start=True, stop=True)
            nc.scalar.activation(g2c[:], zps[:H, :DN], ACTF.Relu, bias=b2d_sb[:, 0:1])
            for tt in range(DN // NP):
                t = b * (DN // NP) + tt
                kps = cpsum_p.tile([128, 500], F32, space="PSUM", tag="cps")
                nc.tensor.transpose(kps[:, :H], g2c[:, tt * NP:(tt + 1) * NP],
                                    ident[:H, :H])
                pv = mk_ap(kps[:], kps[:].offset, [kps[:].ap[0], [1, 1], [1, mposd]])
                nv = mk_ap(kps[:], kps[:].offset + mposd,
                           [kps[:].ap[0], [1, 1], [1, H - mposd]])
                nc.vector.tensor_reduce(kps_t[:, t:t + 1], pv, axis=AX.X, op=OP.add)
                nc.vector.tensor_reduce(kns_t[:, t:t + 1], nv, axis=AX.X, op=OP.add)
        nc.vector.tensor_sub(kraw[:], kps_t[:], kns_t[:])

        kint = dg_p.tile([NP, NT], F32, tag="kint")
        ktmp = dg_p.tile([NP, NT], F32, tag="ktmp")
        nc.vector.tensor_scalar(out=kint[:], in0=kraw[:], scalar1=thr[0], scalar2=None,
                                op0=OP.is_ge)
        for r in range(1, 13):
            nc.vector.tensor_scalar(out=ktmp[:], in0=kraw[:], scalar1=thr[r],
                                    scalar2=None, op0=OP.is_ge)
            nc.vector.tensor_add(kint[:], kint[:], ktmp[:])
        nc.vector.tensor_scalar(out=kint[:], in0=kint[:], scalar1=1.0, scalar2=None,
                                op0=OP.add)
        kc = dg_p.tile([NP, NT], F32, tag="kc")
        nc.scalar.activation(kc[:], kraw[:], ACTF.Sigmoid, bias=db3, scale=1.0)
        nc.vector.tensor_scalar(out=kc[:], in0=kc[:], scalar1=13.0, scalar2=2.0,
                                op0=OP.mult, op1=OP.add)
        nc.sync.dma_start(out_k[:, :], kc[:])

        gate = fin_p.tile([NP, NT * K], F32, tag="gate")
        kiv = kint[:].unsqueeze(2).broadcast_to([NP, NT, K])
        nc.vector.tensor_tensor(out=gate[:].rearrange("p (t j) -> p t j", j=K),
                                in0=kiv, in1=cnt[:].rearrange("p (t j) -> p t j", j=K),
                                op=OP.is_ge)
        nc.sync.dma_start(out_g[:, :], gate[:])
        nc.sync.dma_start(out_e[:, :], E_res[:])
        wpre = fin_p.tile([NP, NT * K], F32, tag="wpre")
        nc.vector.tensor_mul(wpre[:], E_res[:], gate[:])
        den = dg_p.tile([NP, NT], F32, tag="den")
        nc.vector.tensor_reduce(den[:], wpre[:].rearrange("p (t j) -> p t j", j=K),
                                axis=AX.X, op=OP.add)
        nc.vector.tensor_scalar(out=den[:], in0=den[:], scalar1=1e-12, scalar2=None,
                                op0=OP.max)
        rec = dg_p.tile([NP, NT], F32, tag="rec")
        nc.vector.reciprocal(rec[:], den[:])
        rv = rec[:].unsqueeze(2).broadcast_to([NP, NT, K])
        nc.vector.tensor_tensor(out=wpre[:].rearrange("p (t j) -> p t j", j=K),
                                in0=wpre[:].rearrange("p (t j) -> p t j", j=K),
                                in1=rv, op=OP.mult)
        nc.sync.dma_start(out_w[:, :], wpre[:])

    nc.compile()
    return nc


# ---------------- host-side prep ----------------

def prep_meta(ew2, eb2, ew3, eb3, dw2, db2, dw3, db3):
    c3 = np.abs(ew3[:, 0]); s3 = np.sign(ew3[:, 0])
    perm = np.argsort(-s3, kind="stable")
    mpos = int((s3 > 0).sum())
    W2p = (ew2.astype(np.float64) * c3[None, :])[:, perm].astype(np.float32)
    b2p = (eb2.astype(np.float64) * c3)[perm].astype(np.float32)
    cd = np.abs(dw3[:, 0]); sd = np.sign(dw3[:, 0])
    permd = np.argsort(-sd, kind="stable")
    mposd = int((sd > 0).sum())
    W2d = (dw2.astype(np.float64) * cd[None, :])[:, permd].astype(np.float32)
    b2d = (db2.astype(np.float64) * cd)[permd].astype(np.float32)
    wblk = np.zeros((2 * H, 2 * H), np.float32)
    wblk[:H, :H] = W2p
    wblk[H:, H:] = W2p
    b2ps = np.concatenate([b2p, b2p]).reshape(2 * H, 1)
    r = np.arange(3, 16, dtype=np.float64)
    p = (r - 2.5) / 13.0
    thr = (np.log(p / (1 - p)) - float(db3[0])).astype(np.float32)
    return dict(mpos=mpos, mposd=mposd, b3=float(eb3[0]), db3=float(db3[0]),
                thr=thr, W2p=W2p, wblk=wblk, b2ps=b2ps, W2d=W2d,
                b2d=b2d.reshape(H, 1))


def _wrap_idx(idx_tile):
    """idx_tile: [1920] int16 values in gather position order (i = j*128+p).
    Returns [128, 120] int16 in the HW ucode layout."""
    a = np.zeros((128, E_TILE // 16), np.int16)
    i = np.arange(E_TILE)
    a[16 + i % 16, (i // 16) % 8 + 8 * (i // 128)] = idx_tile
    return a


def prep_inputs(inputs):
    x = np.asarray(inputs["x"], np.float32)
    src = np.asarray(inputs["edge_index"][0]).astype(np.int64)
    edist = np.asarray(inputs["edge_dist"], np.float32)
    ew1 = np.asarray(inputs["ew1"], np.float32)
    meta = prep_meta(np.asarray(inputs["ew2"], np.float32),
                     np.asarray(inputs["eb2"], np.float32),
                     np.asarray(inputs["ew3"], np.float32),
                     np.asarray(inputs["eb3"], np.float32),
                     np.asarray(inputs["dw2"], np.float32),
                     np.asarray(inputs["db2"], np.float32),
                     np.asarray(inputs["dw3"], np.float32),
                     np.asarray(inputs["db3"], np.float32))
    xT = np.ascontiguousarray(x.T)
    base = dict(
        wsrc=np.ascontiguousarray(ew1[D:2 * D]),
        wdst=np.ascontiguousarray(ew1[:D]),
        wdist=np.ascontiguousarray(ew1[2 * D:2 * D + 1]),
        eb1=np.asarray(inputs["eb1"], np.float32).reshape(1, H),
        wblk=meta["wblk"], b2p=meta["b2ps"],
        dw1a=np.ascontiguousarray(np.asarray(inputs["dw1"], np.float32)[:D]),
        dw1b=np.ascontiguousarray(np.asarray(inputs["dw1"], np.float32)[D:D + 1]),
        db1=np.asarray(inputs["db1"], np.float32).reshape(1, H),
        w2d=meta["W2d"], b2d=meta["b2d"],
    )
    maps = []
    for c in range(NC_CORES):
        lo = c * N_CORE
        src_c = src[lo * K:(lo + N_CORE) * K].reshape(N_CORE, K)
        dist_c = edist[lo * K:(lo + N_CORE) * K].reshape(N_CORE, K)
        # pad to NPAD nodes
        src_p = np.zeros((NPAD, K), np.int64)
        src_p[:N_CORE] = src_c
        dist_p = np.zeros((NPAD, K), np.float32)
        dist_p[:N_CORE] = dist_c
        m = dict(base)
        # per-half compacted tables
        idx_all = np.zeros((NT, E_TILE), np.int64)
        for half, t0, t1 in ((0, 0, HALF_T), (1, HALF_T, NT)):
            s_half = src_p[t0 * NP:t1 * NP]          # [nodes, K]
            uniq, inv = np.unique(s_half, return_inverse=True)
            nu = len(uniq)
            assert nu <= LH, f"half table overflow: {nu} > {LH}"
            # table row for column q: (q%125)*CH + q//125
            q = np.arange(nu)
            rowof = (q % 125) * CH + q // 125
            rows = rowof[inv.reshape(s_half.shape)]  # [nodes, K]
            # xt columns = x[uniq[q]] for q, padded with zeros
            xt_h = np.zeros((D, LH), np.float32)
            xt_h[:, :nu] = xT[:, uniq]
            m["xta" if half == 0 else "xtb"] = xt_h
            for t in range(t0, t1):
                rt = rows[(t - t0) * NP:(t - t0 + 1) * NP]    # [128, K]
                # gather position i = j*128 + p
                idx_all[t] = rt.T.reshape(-1)
        srcidx = np.zeros((D, NT * (E_TILE // 16)), np.int16)
        for t in range(NT):
            srcidx[:, t * (E_TILE // 16):(t + 1) * (E_TILE // 16)] = \
                _wrap_idx(idx_all[t].astype(np.int16))
        m["srcidx"] = srcidx
        # dist layout [128, t*K+j]
        dist_l = dist_p.reshape(NT, NP, K).transpose(1, 0, 2).reshape(NP, NT * K)
        m["dist"] = np.ascontiguousarray(dist_l)
        xo = np.zeros((D, NPAD), np.float32)
        xo[:, :N_CORE] = xT[:, lo:lo + N_CORE]
        m["xT_own"] = xo
        maps.append(m)
    return maps, meta


def unshard_outputs(results):
    ws, gs, es, ks = [], [], [], []
    for c in range(NC_CORES):
        r = results[c]
        for arr, acc in ((r["out_w"], ws), (r["out_g"], gs), (r["out_e"], es)):
            a = arr.reshape(NP, NT, K).transpose(1, 0, 2).reshape(NPAD * K)
            acc.append(a[:N_CORE * K])
        ks.append(r["out_k"].reshape(NP, NT).T.reshape(NPAD)[:N_CORE])
    return (np.concatenate(ws), np.concatenate(gs),
            np.concatenate(es), np.concatenate(ks))


# ---------------- public entry point ----------------

_CACHED = {}


def kernel(**inputs):
    """Full-input entry: returns (edge_weight, edge_gate, edge_energy, k_cont)
    matching reference.reference(**inputs). Shards across 8 NeuronCores."""
    from concourse.bass_utils import run_bass_kernel_spmd
    maps, meta = prep_inputs(inputs)
    key = (meta["mpos"], meta["mposd"])
    if key not in _CACHED:
        _CACHED[key] = build_kernel(meta)
    nc = _CACHED[key]
    res = run_bass_kernel_spmd(nc, maps, core_ids=list(range(NC_CORES)))
    return unshard_outputs(res.results)
